# revision 5
# baseline (speedup 1.0000x reference)
"""Trainium2 Bass kernel for a 4-layer LIF spiking net scanned over T=32 steps.

Strategy (data-parallel, 8 cores):
  - Shard batch B=2048 -> 256 per core; weights replicated.
  - On-device layout is feature-on-partitions [h, b]: every matmul's
    stationary operand is a static weight tile, spikes are the moving
    operand, so the whole recurrence needs zero on-device transposes.
  - Per layer/step: PE accumulates W@s + bias*1^T into PSUM, ScalarE
    copies out c, PE accumulates beta*I@m into the same PSUM, VectorE
    does m_new = (-thr)*s_prev + psum in one fused scalar_tensor_tensor,
    then s_new = (m_new > thr) via tensor_scalar(is_gt).
    (Note reset_t = H(m_{t-1}-thr) == s_{t-1}, so no extra heaviside.)
  - All output DMAs are fully contiguous 128KB blocks into [T, H, B]
    scratch layout; host transposes back to [T, B, H].
"""

import sys

if "/opt/trn_rl_repo" not in sys.path:
    sys.path.insert(0, "/opt/trn_rl_repo")

import numpy as np

T, B, D, H, D4 = 32, 2048, 48, 256, 10
NCORES = 8
BC = B // NCORES  # 256 batch rows per core
P = 128
WORK_BUFS = 4


def _build(betas, thrs):
    """Build the SPMD Bass program (identical on all cores)."""
    import concourse.mybir as mybir
    from concourse import bacc
    from concourse.tile import TileContext

    f32 = mybir.dt.float32
    Alu = mybir.AluOpType

    # Bacc (not raw Bass): its compile() runs move_matmul_waits_to_ldweights /
    # generate_event_semaphores, which walrus requires (1 sync-wait per inst).
    nc = bacc.Bacc(target_bir_lowering=False)

    # ---- DRAM I/O ----
    xT_d = nc.dram_tensor("xT", [D, T * BC], f32, kind="ExternalInput")
    w1_d = nc.dram_tensor("w1t", [D, H], f32, kind="ExternalInput")
    w2_d = nc.dram_tensor("w2t", [H, H], f32, kind="ExternalInput")
    w3_d = nc.dram_tensor("w3t", [H, H], f32, kind="ExternalInput")
    w4_d = nc.dram_tensor("w4t", [H, D4], f32, kind="ExternalInput")
    b_d = [
        nc.dram_tensor("b1", [1, H], f32, kind="ExternalInput"),
        nc.dram_tensor("b2", [1, H], f32, kind="ExternalInput"),
        nc.dram_tensor("b3", [1, H], f32, kind="ExternalInput"),
        nc.dram_tensor("b4", [1, D4], f32, kind="ExternalInput"),
    ]
    eye_d = [
        nc.dram_tensor("eye1", [P, P], f32, kind="ExternalInput"),
        nc.dram_tensor("eye2", [P, P], f32, kind="ExternalInput"),
        nc.dram_tensor("eye3", [P, P], f32, kind="ExternalInput"),
        nc.dram_tensor("eye4", [D4, D4], f32, kind="ExternalInput"),
    ]
    # outputs in [T, H, B] per-core layout
    so_d, mo_d, co_d = [], [], []
    for l in range(4):
        hl = H if l < 3 else D4
        so_d.append(nc.dram_tensor(f"s{l + 1}o", [T, hl, BC], f32, kind="ExternalOutput"))
        mo_d.append(nc.dram_tensor(f"m{l + 1}o", [T, hl, BC], f32, kind="ExternalOutput"))
        co_d.append(nc.dram_tensor(f"c{l + 1}o", [T, hl, BC], f32, kind="ExternalOutput"))

    with TileContext(nc) as tc:
        with (
            tc.tile_pool(name="const", bufs=1) as cpool,
            tc.tile_pool(name="work", bufs=WORK_BUFS) as wpool,
            tc.tile_pool(name="psum", bufs=8, space="PSUM") as ppool,
        ):
            # ---- load constants ----
            xT_sb = cpool.tile([D, T * BC], f32, name="xT_sb")
            nc.sync.dma_start(xT_sb[:], xT_d[:])
            w1_sb = cpool.tile([D, H], f32, name="w1_sb")
            nc.sync.dma_start(w1_sb[:], w1_d[:])
            w2_sb = [cpool.tile([P, H], f32, name=f"w2_sb{j}") for j in range(2)]
            w3_sb = [cpool.tile([P, H], f32, name=f"w3_sb{j}") for j in range(2)]
            w4_sb = [cpool.tile([P, D4], f32, name=f"w4_sb{j}") for j in range(2)]
            for j in range(2):
                nc.sync.dma_start(w2_sb[j][:], w2_d[j * P : (j + 1) * P, :])
                nc.sync.dma_start(w3_sb[j][:], w3_d[j * P : (j + 1) * P, :])
                nc.sync.dma_start(w4_sb[j][:], w4_d[j * P : (j + 1) * P, :])
            b_sb = []
            for l in range(4):
                hl = H if l < 3 else D4
                t_ = cpool.tile([1, hl], f32, name=f"b_sb{l}")
                nc.sync.dma_start(t_[:], b_d[l][:])
                b_sb.append(t_)
            eye_sb = []
            for l in range(4):
                pl = P if l < 3 else D4
                t_ = cpool.tile([pl, pl], f32, name=f"eye_sb{l}")
                nc.sync.dma_start(t_[:], eye_d[l][:])
                eye_sb.append(t_)
            ones_sb = cpool.tile([1, BC], f32, name="ones_sb")
            nc.vector.memset(ones_sb[:], 1.0)

            # ---- state init ----
            # layer l has ntiles[l] partition tiles of size psz[l]
            ntiles = [2, 2, 2, 1]
            psz = [P, P, P, D4]
            wk = [[w1_sb], w2_sb, w3_sb, w4_sb]  # k-tiles of lhsT per layer
            m_prev = {}
            s_prev = {}
            for l in range(4):
                for tau in range(ntiles[l]):
                    mt = wpool.tile([psz[l], BC], f32, tag=f"m{l}_{tau}", name=f"m{l}_{tau}_i")
                    nc.vector.memset(mt[:], 0.0)
                    m_prev[(l, tau)] = mt
                    st = wpool.tile([psz[l], BC], f32, tag=f"s{l}_{tau}", name=f"s{l}_{tau}_i")
                    nc.vector.memset(st[:], 0.0)
                    s_prev[(l, tau)] = st

            # ---- time loop (fully unrolled) ----
            for t in range(T):
                rhs_tiles = [xT_sb[:, t * BC : (t + 1) * BC]]  # layer-1 moving operand
                for l in range(4):
                    new_s = []
                    for tau in range(ntiles[l]):
                        sl = slice(tau * psz[l], (tau + 1) * psz[l])
                        ps = wpool  # placate linters
                        ps = ppool.tile([psz[l], BC], f32, tag="ps", name=f"ps{l}_{tau}")
                        nk = len(rhs_tiles)
                        for j in range(nk):
                            nc.tensor.matmul(
                                ps[:],
                                wk[l][j][:, sl],
                                rhs_tiles[j],
                                start=(j == 0),
                                stop=False,
                            )
                        # + bias (rank-1: b_l[h] * ones[b])
                        nc.tensor.matmul(ps[:], b_sb[l][:, sl], ones_sb[:], start=False, stop=False)
                        # c output (PSUM -> SBUF) before recurrent terms pollute PSUM
                        c = wpool.tile([psz[l], BC], f32, tag=f"c{l}_{tau}", name=f"c{l}_{tau}")
                        nc.scalar.copy(c[:], ps[:])
                        # += beta_l * m_prev
                        nc.tensor.matmul(ps[:], eye_sb[l][:], m_prev[(l, tau)][:], start=False, stop=True)
                        # m_new = (-thr)*s_prev + psum
                        m = wpool.tile([psz[l], BC], f32, tag=f"m{l}_{tau}", name=f"m{l}_{tau}")
                        nc.vector.scalar_tensor_tensor(
                            m[:], s_prev[(l, tau)][:], -thrs[l], ps[:], Alu.mult, Alu.add
                        )
                        # s_new = (m_new > thr)
                        s = wpool.tile([psz[l], BC], f32, tag=f"s{l}_{tau}", name=f"s{l}_{tau}")
                        nc.vector.tensor_scalar(s[:], m[:], thrs[l], None, Alu.is_gt)
                        # stream outputs (each fully contiguous in HBM)
                        nc.sync.dma_start(co_d[l][t, sl, :], c[:])
                        nc.sync.dma_start(mo_d[l][t, sl, :], m[:])
                        nc.sync.dma_start(so_d[l][t, sl, :], s[:])
                        m_prev[(l, tau)] = m
                        s_prev[(l, tau)] = s
                        new_s.append(s)
                    rhs_tiles = [sb[:] for sb in new_s]

    nc.compile()
    return nc


LAST = None  # last BassKernelResults (for test harness: exec_time_ns, trace)


def kernel(**inputs):
    import os

    from concourse.bass_utils import run_bass_kernel_spmd

    x = np.asarray(inputs["x"], np.float32)
    Ws = [np.asarray(inputs[f"W{i}"], np.float32) for i in (1, 2, 3, 4)]
    bs = [np.asarray(inputs[f"b{i}"], np.float32) for i in (1, 2, 3, 4)]
    betas = [float(np.clip(np.float32(inputs[f"beta{i}"]), 0.0, 1.0)) for i in (1, 2, 3, 4)]
    thrs = [float(np.float32(inputs[f"thr{i}"])) for i in (1, 2, 3, 4)]

    nc = _build(betas, thrs)

    # shared (replicated) inputs
    shared = {
        "w1t": np.ascontiguousarray(Ws[0].T),
        "w2t": np.ascontiguousarray(Ws[1].T),
        "w3t": np.ascontiguousarray(Ws[2].T),
        "w4t": np.ascontiguousarray(Ws[3].T),
        "b1": bs[0].reshape(1, H),
        "b2": bs[1].reshape(1, H),
        "b3": bs[2].reshape(1, H),
        "b4": bs[3].reshape(1, D4),
        "eye1": (betas[0] * np.eye(P, dtype=np.float32)),
        "eye2": (betas[1] * np.eye(P, dtype=np.float32)),
        "eye3": (betas[2] * np.eye(P, dtype=np.float32)),
        "eye4": (betas[3] * np.eye(D4, dtype=np.float32)),
    }
    in_maps = []
    for c in range(NCORES):
        xc = x[c * BC : (c + 1) * BC]  # [BC, T, D]
        xT = np.ascontiguousarray(xc.transpose(2, 1, 0).reshape(D, T * BC))
        m = dict(shared)
        m["xT"] = xT
        in_maps.append(m)

    kwargs = {}
    if os.environ.get("KTRACE"):
        kwargs["trace"] = True
        if os.environ.get("KTRACE_DIR"):
            kwargs["tmpdir"] = os.environ["KTRACE_DIR"]
    res = run_bass_kernel_spmd(nc, in_maps, core_ids=list(range(NCORES)), **kwargs)
    global LAST
    LAST = res
    results = res.results

    outs = []
    for kind in ("s", "m", "c"):
        for l in range(4):
            hl = H if l < 3 else D4
            full = np.empty((T, B, hl), np.float32)
            for c in range(NCORES):
                dev = results[c][f"{kind}{l + 1}o"]  # [T, hl, BC]
                full[:, c * BC : (c + 1) * BC, :] = dev.transpose(0, 2, 1)
            outs.append(full)
    # reference order: (s1..s4, m1..m4, c1..c4)
    return tuple(outs)


if __name__ == "__main__":
    pass
